# revision 28
# baseline (speedup 1.0000x reference)
"""Trainium2 Bass kernel for ClassicPINN forward pass (15-layer tiny MLP, tanh).

Strategy (v3)
-------------
v3 on top of v2 (HW-microbenchmarked): the pipeline is bound by the
ScalarE ACTIVATE stream, whose cost is ~(FD/2)/1.2GHz + ~0.45us fixed
per instruction. With PSUM capping tiles at [128, 2048] x 2, the
instruction count is fixed, so v3 cuts per-instruction cost instead:
  * Activations of L0-L9 (and the matching L1-L10 weights) are fp8e4m3:
    an fp8-out ACTIVATE measures ~0.19us faster per [128,2048] drain,
    and the early layers' quantization noise contracts away through the
    remaining stack (numpy: 4.9e-3 vs 3.0e-3 pure-bf16, tol 2e-2).
  * The final 16->3 layer packs its output 4-wide (pad 1) instead of
    8-wide: stack=4 col-tiled matmuls (explicit tile_position - the
    auto-derive rejects base partition 96), a [128,1024] 2-bank mega,
    half the drain cols and half the output-store bytes of v2.
  * L14_ON_DVE optionally moves that tanh-free drain to the idle DVE;
    it defaults off - the PE->DVE->PE PSUM-slot chain it creates can
    starve ACT at pair boundaries (the v1 regression) and the A/B was
    within timing noise.

Strategy (v2)
-------------
Pure data parallel over 8 NeuronCores (131072 points each); 4 chunks of
32768 points per core. Activations live feature-on-partition in bf16:
the 128 SBUF partitions hold G groups of the layer width, each group a
different 512-point column block. Weights are block-diagonalized on the
host so every matmul is dense [K<=128, M<=128] x [K, 512] -> PSUM fp32.

ACT (ScalarE tanh) is the roofline (~216 tanh elems/point). v2 cuts ACT
waste and idle vs v1:
  * bf16 matmul dataflow (same 1 cyc/row PE speed as fp32r) allows PE
    tile_position, so layers with Mmm=64 (L13, L14) stack 2 matmuls
    partition-wise in one PSUM bank set -> every ACT drains a full
    [128, 2048] tile (v1 ran half-width ACTs for L13).
  * L14 reads the *folded* L13 tile with Gmm=8 (was 4): half the
    matmuls and half the DVE drain of v1.
  * All 15 layers run in interleaved chunk pairs (bf16 halves SBUF so
    the wide layers fit); each pair's L14 is deferred into the next
    pair's early layers and the next pair's L0 is pre-emitted during
    L13, so ACT never waits on a pair transition.
  * The L14 bias-add runs on ACT as Identity (every tanh act-table set
    contains identity, so no table reload): a DVE drain would expose a
    cross-engine PE->DVE->PE PSUM-reuse stall on ACT's critical path.
  * Coords prefetched a pair ahead; weight DMA split so L0/L1 weights
    land before the bulk; bf16 output halves the out-DMA; dummy matmuls
    at t0 ramp the PE p-state while the first DMAs are in flight; the
    program-final L14s drain in halves so the store DMA overlaps.

The same schedule object drives the Bass builder, a numpy simulator
(used by test.py), and an integer "point id" replay that yields the
output unpack permutation.
"""

import numpy as np
from contextlib import ExitStack
import ml_dtypes

BF16 = ml_dtypes.bfloat16
F8 = ml_dtypes.float8_e4m3fn

WIDTHS = [3, 8, 8, 8, 8, 8, 8, 8, 16, 16, 16, 32, 32, 32, 16, 3]
N_LAYERS = 15
N_POINTS = 1048576
NCORES = 8
PPC = N_POINTS // NCORES          # 131072 points per core
NCHUNKS = 4
CHUNK = PPC // NCHUNKS            # 32768 points per chunk
FREE = 512                        # matmul moving free dim
BANKS = 4                         # PSUM banks per mega-tile / ACT
BLK = BANKS * FREE                # 2048 cols per mega-tile
WT_HEAD_LAYERS = 2                # layers whose weights ship in the early DMA
# Layers whose tanh output is stored as fp8e4m3 (and whose consumers'
# weights are fp8): the early narrow layers tolerate it freely (numpy
# check: rel err 2.7e-3 vs 3.0e-3 pure-bf16) and fp8 ACT output may run
# the ScalarE drain at 4x. Empty set disables the path.
FP8_ACT_LAYERS = frozenset(range(10))
# Drain the final (tanh-free) layer on DVE instead of ACT. Saves ~0.8us
# of ACT per chunk but inserts a PE->DVE->PE PSUM-slot chain that can
# starve ACT at pair boundaries (the v1 regression); measure both.
L14_ON_DVE = False


class _Layer:
    pass


def _make_schedule():
    """Uniform per-layer schedule. Every layer's input is a physical
    [P_in, C_in] tile holding G_in groups of in_w features per column
    (stacked layers interleave two virtual column blocks row-wise, which
    simply doubles the group count seen by the next layer)."""
    layers = []
    G_in, C_in, P_in = 16, 2048, 48   # coords: 16 groups x 3 feats
    w_off = 0
    for i in range(N_LAYERS):
        in_w, out_w = WIDTHS[i], WIDTHS[i + 1]
        out_w_pad = 4 if out_w == 3 else out_w
        L = _Layer()
        L.i, L.in_w, L.out_w, L.out_w_pad = i, in_w, out_w, out_w_pad
        L.G_in, L.C_in, L.P_in = G_in, C_in, P_in
        L.Gmm = min(G_in, 128 // out_w_pad)
        L.n_half = G_in // L.Gmm
        L.Kmm = L.Gmm * in_w
        L.Mmm = L.Gmm * out_w_pad
        L.stack = 128 // L.Mmm        # matmuls stacked per PSUM bank set
        L.ncb = C_in // FREE
        L.n_mms = L.ncb * L.n_half
        # Last layer packs 4-wide output groups 4-deep per bank: its one
        # mega-tile spans only 2 banks ([128, 1024]).
        L.banks = min(BANKS, L.n_mms // L.stack)
        L.blk = L.banks * FREE
        assert L.n_mms % (L.banks * L.stack) == 0, (i, L.n_mms, L.stack)
        L.n_mega = L.n_mms // (L.banks * L.stack)
        L.C_out = L.n_mega * L.blk
        L.w_off = w_off
        w_off += L.Mmm
        layers.append(L)
        G_in, C_in, P_in = 128 // out_w_pad, L.C_out, 128
    return layers, w_off


_LAYERS, W_TOTAL = _make_schedule()


def _assign_weight_groups():
    """Partition layer weights into dram tensors by dtype/urgency.

    Layer i's weights must match its input dtype: fp8 iff layer i-1's
    activation output is fp8. Groups: "head" (early bf16, L0/L1),
    "f8" (all fp8 layers, also shipped early), "rest" (late bf16).
    """
    cols = {"head": 0, "f8": 0, "rest": 0}
    for L in _LAYERS:
        fp8_w = (L.i - 1) in FP8_ACT_LAYERS
        if fp8_w:
            L.w_group = "f8"
        elif L.i < WT_HEAD_LAYERS:
            L.w_group = "head"
        else:
            L.w_group = "rest"
        L.w_col = cols[L.w_group]
        cols[L.w_group] += L.Mmm
    return cols


_W_COLS = _assign_weight_groups()


def _mm_geom(L, m):
    """Matmul m -> (input half, input col block, mega-tile, stack row, bank)."""
    h, cbi = divmod(m, L.ncb)
    tau, r = divmod(m, L.banks * L.stack)
    s, b = divmod(r, L.banks)
    return h, cbi, tau, s, b


# ---------------------------------------------------------------- host packing

def pack_coords(coords):
    """[N_POINTS, 3] -> bf16 [NCORES, NCHUNKS, 48, 2048] matching L0 layout.

    Per core: point n = chunk*32768 + t*8192 + g*512 + j lives at
    partition g*3+f, column t*512+j of tile [core, chunk].
    """
    c = np.ascontiguousarray(coords, dtype=np.float32)
    c = c.reshape(NCORES, NCHUNKS, 4, 16, FREE, 3)
    c = c.transpose(0, 1, 3, 5, 2, 4)            # core, chunk, g, f, t, j
    return np.ascontiguousarray(c.reshape(NCORES, NCHUNKS, 48, 2048)).astype(BF16)


def build_weights(Ws, bs):
    """Block-diagonal lhsT stacks per dtype group and bias matrix [128, 15]."""
    stacks = {k: np.zeros((128, n), np.float32) for k, n in _W_COLS.items()}
    biases = np.zeros((128, N_LAYERS), np.float32)
    for L in _LAYERS:
        W = np.asarray(Ws[L.i], np.float32)      # [out_w, in_w]
        bd = np.zeros((L.Kmm, L.Mmm), np.float32)
        for g in range(L.Gmm):
            bd[g * L.in_w:(g + 1) * L.in_w,
               g * L.out_w_pad:g * L.out_w_pad + L.out_w] = W.T
        for h in range(L.n_half):
            base = h * L.Kmm
            stacks[L.w_group][base:base + L.Kmm,
                              L.w_col:L.w_col + L.Mmm] = bd
        b = np.asarray(bs[L.i], np.float32)
        q = np.arange(128) % L.out_w_pad
        col = np.where(q < L.out_w, b[np.minimum(q, L.out_w - 1)], 0.0)
        biases[:, L.i] = col
    out = {"lhsT_head": stacks["head"].astype(BF16),
           "lhsT_rest": stacks["rest"].astype(BF16),
           "biases": biases}
    if _W_COLS["f8"]:
        out["lhsT_f8"] = stacks["f8"].astype(F8)
    return out


def replay_ids():
    """Propagate chunk-local point ids through the schedule.

    Returns [128, 2048] int array: element (p, c) of the final output
    tile holds component (p % 8) of chunk-local point ids[p, c].
    """
    ids = np.zeros((48, 2048), np.int64)
    j = np.arange(FREE)
    for g in range(16):
        for t in range(4):
            for f in range(3):
                ids[g * 3 + f, t * FREE:(t + 1) * FREE] = t * 8192 + g * FREE + j
    for L in _LAYERS:
        out = np.zeros((128, L.C_out), np.int64)
        for m in range(L.n_mms):
            h, cbi, tau, s, b = _mm_geom(L, m)
            src = ids[h * L.Kmm:h * L.Kmm + L.Kmm:L.in_w,
                      cbi * FREE:(cbi + 1) * FREE]          # [Gmm, 512]
            out[s * L.Mmm:(s + 1) * L.Mmm,
                tau * L.blk + b * FREE:tau * L.blk + (b + 1) * FREE] = \
                np.repeat(src, L.out_w_pad, axis=0)
        ids = out
    return ids


def simulate_chunk(coords_tile, weights):
    """Numpy mirror of the device program for one [48, 2048] chunk tile."""
    group_of = {"head": "lhsT_head", "f8": "lhsT_f8", "rest": "lhsT_rest"}
    biases = np.asarray(weights["biases"], np.float32)
    act = np.asarray(coords_tile, np.float32)
    for L in _LAYERS:
        stack = np.asarray(weights[group_of[L.w_group]], np.float32)
        out = np.zeros((128, L.C_out), np.float32)
        for m in range(L.n_mms):
            h, cbi, tau, s, b = _mm_geom(L, m)
            lhsT = stack[h * L.Kmm:h * L.Kmm + L.Kmm,
                         L.w_col:L.w_col + L.Mmm]
            rhs = act[h * L.Kmm:h * L.Kmm + L.Kmm,
                      cbi * FREE:(cbi + 1) * FREE]
            out[s * L.Mmm:(s + 1) * L.Mmm,
                tau * L.blk + b * FREE:tau * L.blk + (b + 1) * FREE] = \
                lhsT.T @ rhs
        out += biases[:, L.i:L.i + 1]
        act = np.tanh(out) if L.i < N_LAYERS - 1 else out
        dt = F8 if L.i in FP8_ACT_LAYERS else BF16
        act = act.astype(dt).astype(np.float32)
    return act                                   # [128, C_out(last)]


def unpack_output(per_core_out):
    """[NCORES][NCHUNKS, 128, 2048] device tiles -> [N_POINTS, 3]."""
    ids = replay_ids()
    rows = np.arange(128)
    comp = rows % _LAYERS[-1].out_w_pad
    valid = comp < 3
    n_idx = ids[valid]
    o_idx = np.broadcast_to(comp[valid][:, None], n_idx.shape)
    out = np.empty((N_POINTS, 3), np.float32)
    for core in range(NCORES):
        tiles = per_core_out[core]
        for chunk in range(NCHUNKS):
            base = core * PPC + chunk * CHUNK
            t = np.asarray(tiles[chunk], np.float32)
            out[base + n_idx, o_idx] = t[valid]
    return out


# ---------------------------------------------------------------- bass program

_PROGRAM_CACHE = {}


def _build_program(repeat=1):
    import concourse.bacc as bacc
    import concourse.tile as tile
    from concourse import mybir

    nc = bacc.Bacc("TRN2", target_bir_lowering=False, debug=False,
                   enable_asserts=False, num_devices=NCORES)
    f32 = mybir.dt.float32
    b16 = mybir.dt.bfloat16
    f8 = mybir.dt.float8e4
    coords_d = nc.dram_tensor("coords", (NCHUNKS, 48, 2048), b16,
                              kind="ExternalInput").ap()
    wh_d = nc.dram_tensor("lhsT_head", (128, _W_COLS["head"]), b16,
                          kind="ExternalInput").ap()
    wr_d = nc.dram_tensor("lhsT_rest", (128, _W_COLS["rest"]), b16,
                          kind="ExternalInput").ap()
    wf8_d = (nc.dram_tensor("lhsT_f8", (128, _W_COLS["f8"]), f8,
                            kind="ExternalInput").ap()
             if _W_COLS["f8"] else None)
    b_d = nc.dram_tensor("biases", (128, N_LAYERS), f32,
                         kind="ExternalInput").ap()
    out_d = nc.dram_tensor("out", (NCHUNKS, 128, _LAYERS[-1].C_out), b16,
                           kind="ExternalOutput").ap()

    TANH = mybir.ActivationFunctionType.Tanh
    IDENT = mybir.ActivationFunctionType.Identity

    with tile.TileContext(nc) as tc, ExitStack() as ctx:
        wpool = ctx.enter_context(tc.tile_pool(name="weights", bufs=1))
        cpool = ctx.enter_context(tc.tile_pool(name="cin", bufs=6))
        pA = ctx.enter_context(tc.tile_pool(name="a2k", bufs=4))
        pB = ctx.enter_context(tc.tile_pool(name="a4k", bufs=6))
        pC = ctx.enter_context(tc.tile_pool(name="a8k", bufs=4))
        pout = ctx.enter_context(tc.tile_pool(name="aout", bufs=4))
        pspool = ctx.enter_context(
            tc.tile_pool(name="psum", bufs=2, space="PSUM"))

        # PE p-state warmup: the tensor engine needs ~3us of continuous
        # execution to reach full clock. Run throwaway matmuls on a
        # zeroed scratch tile while the input DMAs are in flight so L0
        # hits the PE at full speed. GPSIMD does the memset (every other
        # engine has real work at t0).
        dummy = wpool.tile([128, FREE], b16, tag="warm")
        nc.gpsimd.memset(dummy[:], 0.0)
        wps = pspool.tile([128, BLK], f32, tag="ps")
        for _ in range(6):
            nc.tensor.matmul(wps[:, 0:FREE], dummy[:, 0:128], dummy[:],
                             start=True, stop=True)
        # Dummy tanh at t0: the act-table load (~1.3us) is otherwise
        # charged lazily against the first real activation. ACT is idle
        # here, so the table is hot before L0's tanh arrives.
        warm_act = wpool.tile([128, FREE], b16, tag="warmact")
        nc.scalar.activation(warm_act[:], dummy[:], TANH)

        wt_head = wpool.tile([128, _W_COLS["head"]], b16, tag="wth")
        nc.sync.dma_start(out=wt_head[:], in_=wh_d[:])
        bt = wpool.tile([128, N_LAYERS], f32, tag="bt")
        nc.sync.dma_start(out=bt[:], in_=b_d[:])

        ct_of = {}

        def fetch(c):
            t = cpool.tile([48, 2048], b16, tag="cin")
            nc.sync.dma_start(out=t[:], in_=coords_d[c % NCHUNKS])
            ct_of[c] = t

        pool_by_cols = {2048: pA, 4096: pB, 8192: pC}

        def wslice(L, h):
            wt = {"head": wt_head, "f8": wt_f8, "rest": wt_rest}[L.w_group]
            return wt[h * L.Kmm:(h + 1) * L.Kmm, L.w_col:L.w_col + L.Mmm]

        def emit_layer(L, act):
            is_last = L.i == N_LAYERS - 1
            pool = pout if is_last else pool_by_cols[L.C_out]
            out_dt = f8 if L.i in FP8_ACT_LAYERS else b16
            out_t = pool.tile([128, L.C_out], out_dt, tag=pool.name)
            for tau in range(L.n_mega):
                ps = pspool.tile([128, L.blk], f32, tag="ps")
                for r in range(L.banks * L.stack):
                    m = tau * L.banks * L.stack + r
                    h, cbi, _, s, b = _mm_geom(L, m)
                    nc.tensor.matmul(
                        ps[s * L.Mmm:(s + 1) * L.Mmm,
                           b * FREE:(b + 1) * FREE],
                        wslice(L, h),
                        act[h * L.Kmm:(h + 1) * L.Kmm,
                            cbi * FREE:(cbi + 1) * FREE],
                        start=True, stop=True,
                        # auto-derive rejects the 4th col slot (96)
                        tile_position=(h * L.Kmm % 128, s * L.Mmm % 128))
                dst = out_t[:, tau * L.blk:(tau + 1) * L.blk]
                if is_last and L14_ON_DVE:
                    # Two halves so each releases its PSUM banks sooner,
                    # softening the PE->DVE->PE slot-return stall.
                    half = L.blk // 2
                    for j in range(2):
                        nc.vector.tensor_scalar_add(
                            dst[:, j * half:(j + 1) * half],
                            ps[:, j * half:(j + 1) * half],
                            bt[:, L.i:L.i + 1])
                elif is_last:
                    nc.scalar.activation(
                        dst, ps[:], IDENT, bias=bt[:, L.i:L.i + 1])
                else:
                    nc.scalar.activation(
                        dst, ps[:], TANH, bias=bt[:, L.i:L.i + 1])
            return out_t

        def emit_tail(c, acts, split=False):
            if not split:
                out_t = emit_layer(_LAYERS[-1], acts[c])
                nc.sync.dma_start(out=out_d[c % NCHUNKS], in_=out_t[:])
                return
            # Program-final tails: drain the single L14 mega in halves
            # into separate tiles so the first half's store DMA overlaps
            # the second half's drain.
            L = _LAYERS[-1]
            ps = pspool.tile([128, L.blk], f32, tag="ps")
            for m in range(L.banks * L.stack):
                h, cbi, _, s, b = _mm_geom(L, m)
                nc.tensor.matmul(
                    ps[s * L.Mmm:(s + 1) * L.Mmm, b * FREE:(b + 1) * FREE],
                    wslice(L, h),
                    acts[c][h * L.Kmm:(h + 1) * L.Kmm,
                            cbi * FREE:(cbi + 1) * FREE],
                    start=True, stop=True,
                    tile_position=(h * L.Kmm % 128, s * L.Mmm % 128))
            half = L.blk // 2
            for j in range(2):
                ot = pout.tile([128, half], b16, tag="aout")
                if L14_ON_DVE:
                    nc.vector.tensor_scalar_add(
                        ot[:], ps[:, j * half:(j + 1) * half],
                        bt[:, L.i:L.i + 1])
                else:
                    nc.scalar.activation(
                        ot[:], ps[:, j * half:(j + 1) * half], IDENT,
                        bias=bt[:, L.i:L.i + 1])
                nc.sync.dma_start(
                    out=out_d[c % NCHUNKS][:, j * half:(j + 1) * half],
                    in_=ot[:])

        seq = [r * NCHUNKS + c for r in range(repeat) for c in range(NCHUNKS)]
        pairs = list(zip(seq[0::2], seq[1::2]))

        # Coords for the first two pairs land before the weight bulk so
        # L0 can start as early as possible. fp8 weights (early layers)
        # ship right after the first coords tiles.
        fetch(pairs[0][0])
        if _W_COLS["f8"]:
            wt_f8 = wpool.tile([128, _W_COLS["f8"]], f8, tag="wtf8")
            nc.sync.dma_start(out=wt_f8[:], in_=wf8_d[:])
        else:
            wt_f8 = None
        fetch(pairs[0][1])
        wt_rest = wpool.tile([128, _W_COLS["rest"]], b16, tag="wtr")
        nc.sync.dma_start(out=wt_rest[:], in_=wr_d[:])
        if len(pairs) > 1:
            fetch(pairs[1][0])
            fetch(pairs[1][1])

        acts = {}
        prev = None
        for pi, (ca, cb) in enumerate(pairs):
            is_last_pair = pi == len(pairs) - 1
            first_li = 0
            if pi == 0:
                acts[ca] = ct_of.pop(ca)
                acts[cb] = ct_of.pop(cb)
            else:
                first_li = 1      # L0 was pre-emitted by the previous pair
            for li in range(first_li, N_LAYERS - 1):
                L = _LAYERS[li]
                for ci, c in enumerate((ca, cb)):
                    acts[c] = emit_layer(L, acts[c])
                    if li == N_LAYERS - 2 and not is_last_pair:
                        # Keep ACT fed across the pair boundary: the
                        # next pair's L0 goes onto the PE queue now.
                        n = pairs[pi + 1][ci]
                        acts[n] = emit_layer(_LAYERS[0], ct_of.pop(n))
                if li == N_LAYERS - 2 and is_last_pair:
                    for c in (ca, cb):
                        emit_tail(c, acts, split=True)
                if li == 1:
                    # ACT is busy with L0/L1 tanh here; slot the previous
                    # pair's (ACT-free) final layer into the PE stream now
                    # so pair transitions cost ACT nothing.
                    if prev is not None:
                        for c in prev:
                            emit_tail(c, acts)
                    if pi + 2 < len(pairs):
                        fetch(pairs[pi + 2][0])
                        fetch(pairs[pi + 2][1])
            prev = (ca, cb)

    nc.compile()
    return nc


def get_program(repeat=1):
    key = ("nc", repeat)
    if key not in _PROGRAM_CACHE:
        _PROGRAM_CACHE[key] = _build_program(repeat)
    return _PROGRAM_CACHE[key]


def make_in_maps(coords, Ws, bs):
    cp = pack_coords(coords)
    weights = build_weights(Ws, bs)
    return [{"coords": cp[core], **weights} for core in range(NCORES)]


def kernel(**inputs):
    from concourse.bass_utils import run_bass_kernel_spmd

    coords = np.asarray(inputs["coords"], np.float32)
    Ws = [np.asarray(inputs[f"W{i}"], np.float32) for i in range(N_LAYERS)]
    bs = [np.asarray(inputs[f"b{i}"], np.float32) for i in range(N_LAYERS)]

    nc = get_program()
    in_maps = make_in_maps(coords, Ws, bs)
    res = run_bass_kernel_spmd(nc, in_maps, list(range(NCORES)))
    per_core = [res.results[c]["out"] for c in range(NCORES)]
    full = unpack_output(per_core)
    return (full[:, 0:1], full[:, 1:2], full[:, 2:3])



# revision 31
# speedup vs baseline: 1.1231x; 1.1231x over previous
"""Trainium2 Bass kernel for ClassicPINN forward pass (15-layer tiny MLP, tanh).

Strategy (v3)
-------------
v3 on top of v2 (HW-microbenchmarked): the pipeline is bound by the
ScalarE ACTIVATE stream, whose cost is ~(FD/2)/1.2GHz + ~0.45us fixed
per instruction. With PSUM capping tiles at [128, 2048] x 2, the
instruction count is fixed, so v3 cuts per-instruction cost instead:
  * Activations of L0-L9 (and the matching L1-L10 weights) are fp8e4m3:
    an fp8-out ACTIVATE measures ~0.19us faster per [128,2048] drain,
    and the early layers' quantization noise contracts away through the
    remaining stack (numpy: 4.9e-3 vs 3.0e-3 pure-bf16, tol 2e-2).
  * The final 16->3 layer packs its output 4-wide (pad 1) instead of
    8-wide: stack=4 col-tiled matmuls (explicit tile_position - the
    auto-derive rejects base partition 96), a [128,1024] 2-bank mega,
    half the drain cols and half the output-store bytes of v2.
  * L14_ON_DVE optionally moves that tanh-free drain to the idle DVE;
    it defaults off - the PE->DVE->PE PSUM-slot chain it creates can
    starve ACT at pair boundaries (the v1 regression) and the A/B was
    within timing noise.

Strategy (v2)
-------------
Pure data parallel over 8 NeuronCores (131072 points each); 4 chunks of
32768 points per core. Activations live feature-on-partition in bf16:
the 128 SBUF partitions hold G groups of the layer width, each group a
different 512-point column block. Weights are block-diagonalized on the
host so every matmul is dense [K<=128, M<=128] x [K, 512] -> PSUM fp32.

ACT (ScalarE tanh) is the roofline (~216 tanh elems/point). v2 cuts ACT
waste and idle vs v1:
  * bf16 matmul dataflow (same 1 cyc/row PE speed as fp32r) allows PE
    tile_position, so layers with Mmm=64 (L13, L14) stack 2 matmuls
    partition-wise in one PSUM bank set -> every ACT drains a full
    [128, 2048] tile (v1 ran half-width ACTs for L13).
  * L14 reads the *folded* L13 tile with Gmm=8 (was 4): half the
    matmuls and half the DVE drain of v1.
  * All 15 layers run in interleaved chunk pairs (bf16 halves SBUF so
    the wide layers fit); each pair's L14 is deferred into the next
    pair's early layers and the next pair's L0 is pre-emitted during
    L13, so ACT never waits on a pair transition.
  * The L14 bias-add runs on ACT as Identity (every tanh act-table set
    contains identity, so no table reload): a DVE drain would expose a
    cross-engine PE->DVE->PE PSUM-reuse stall on ACT's critical path.
  * Coords prefetched a pair ahead; weight DMA split so L0/L1 weights
    land before the bulk; bf16 output halves the out-DMA; dummy matmuls
    at t0 ramp the PE p-state while the first DMAs are in flight; the
    program-final L14s drain in halves so the store DMA overlaps.

The same schedule object drives the Bass builder, a numpy simulator
(used by test.py), and an integer "point id" replay that yields the
output unpack permutation.
"""

import numpy as np
from contextlib import ExitStack
import ml_dtypes

BF16 = ml_dtypes.bfloat16
F8 = ml_dtypes.float8_e4m3fn

WIDTHS = [3, 8, 8, 8, 8, 8, 8, 8, 16, 16, 16, 32, 32, 32, 16, 3]
N_LAYERS = 15
N_POINTS = 1048576
NCORES = 8
PPC = N_POINTS // NCORES          # 131072 points per core
NCHUNKS = 4
CHUNK = PPC // NCHUNKS            # 32768 points per chunk
FREE = 512                        # matmul moving free dim
BANKS = 4                         # PSUM banks per mega-tile / ACT
BLK = BANKS * FREE                # 2048 cols per mega-tile
WT_HEAD_LAYERS = 2                # layers whose weights ship in the early DMA
# Layers whose tanh output is stored as fp8e4m3 (and whose consumers'
# weights are fp8): the early narrow layers tolerate it freely (numpy
# check: rel err 2.7e-3 vs 3.0e-3 pure-bf16) and fp8 ACT output may run
# the ScalarE drain at 4x. Empty set disables the path.
FP8_ACT_LAYERS = frozenset(range(11))
# Drain the final (tanh-free) layer on DVE instead of ACT. Saves ~0.8us
# of ACT per chunk but inserts a PE->DVE->PE PSUM-slot chain that can
# starve ACT at pair boundaries (the v1 regression); measure both.
L14_ON_DVE = False


class _Layer:
    pass


def _make_schedule():
    """Uniform per-layer schedule. Every layer's input is a physical
    [P_in, C_in] tile holding G_in groups of in_w features per column
    (stacked layers interleave two virtual column blocks row-wise, which
    simply doubles the group count seen by the next layer)."""
    layers = []
    G_in, C_in, P_in = 16, 2048, 48   # coords: 16 groups x 3 feats
    w_off = 0
    for i in range(N_LAYERS):
        in_w, out_w = WIDTHS[i], WIDTHS[i + 1]
        out_w_pad = 4 if out_w == 3 else out_w
        L = _Layer()
        L.i, L.in_w, L.out_w, L.out_w_pad = i, in_w, out_w, out_w_pad
        L.G_in, L.C_in, L.P_in = G_in, C_in, P_in
        L.Gmm = min(G_in, 128 // out_w_pad)
        L.n_half = G_in // L.Gmm
        L.Kmm = L.Gmm * in_w
        L.Mmm = L.Gmm * out_w_pad
        L.stack = 128 // L.Mmm        # matmuls stacked per PSUM bank set
        L.ncb = C_in // FREE
        L.n_mms = L.ncb * L.n_half
        # Last layer packs 4-wide output groups 4-deep per bank: its one
        # mega-tile spans only 2 banks ([128, 1024]).
        L.banks = min(BANKS, L.n_mms // L.stack)
        L.blk = L.banks * FREE
        assert L.n_mms % (L.banks * L.stack) == 0, (i, L.n_mms, L.stack)
        L.n_mega = L.n_mms // (L.banks * L.stack)
        L.C_out = L.n_mega * L.blk
        L.w_off = w_off
        w_off += L.Mmm
        layers.append(L)
        G_in, C_in, P_in = 128 // out_w_pad, L.C_out, 128
    return layers, w_off


_LAYERS, W_TOTAL = _make_schedule()


def _assign_weight_groups():
    """Partition layer weights into dram tensors by dtype/urgency.

    Layer i's weights must match its input dtype: fp8 iff layer i-1's
    activation output is fp8. Groups: "head" (early bf16, L0/L1),
    "f8" (all fp8 layers, also shipped early), "rest" (late bf16).
    """
    cols = {"head": 0, "f8": 0, "rest": 0}
    for L in _LAYERS:
        fp8_w = (L.i - 1) in FP8_ACT_LAYERS
        if fp8_w:
            L.w_group = "f8"
        elif L.i < WT_HEAD_LAYERS:
            L.w_group = "head"
        else:
            L.w_group = "rest"
        L.w_col = cols[L.w_group]
        cols[L.w_group] += L.Mmm
    return cols


_W_COLS = _assign_weight_groups()


def _mm_geom(L, m):
    """Matmul m -> (input half, input col block, mega-tile, stack row, bank)."""
    h, cbi = divmod(m, L.ncb)
    tau, r = divmod(m, L.banks * L.stack)
    s, b = divmod(r, L.banks)
    return h, cbi, tau, s, b


# ---------------------------------------------------------------- host packing

def pack_coords(coords):
    """[N_POINTS, 3] -> bf16 [NCORES, NCHUNKS, 48, 2048] matching L0 layout.

    Per core: point n = chunk*32768 + t*8192 + g*512 + j lives at
    partition g*3+f, column t*512+j of tile [core, chunk].
    """
    c = np.ascontiguousarray(coords, dtype=np.float32)
    c = c.reshape(NCORES, NCHUNKS, 4, 16, FREE, 3)
    c = c.transpose(0, 1, 3, 5, 2, 4)            # core, chunk, g, f, t, j
    return np.ascontiguousarray(c.reshape(NCORES, NCHUNKS, 48, 2048)).astype(BF16)


def build_weights(Ws, bs):
    """Block-diagonal lhsT stacks per dtype group and bias matrix [128, 15]."""
    stacks = {k: np.zeros((128, n), np.float32) for k, n in _W_COLS.items()}
    biases = np.zeros((128, N_LAYERS), np.float32)
    for L in _LAYERS:
        W = np.asarray(Ws[L.i], np.float32)      # [out_w, in_w]
        bd = np.zeros((L.Kmm, L.Mmm), np.float32)
        for g in range(L.Gmm):
            bd[g * L.in_w:(g + 1) * L.in_w,
               g * L.out_w_pad:g * L.out_w_pad + L.out_w] = W.T
        for h in range(L.n_half):
            base = h * L.Kmm
            stacks[L.w_group][base:base + L.Kmm,
                              L.w_col:L.w_col + L.Mmm] = bd
        b = np.asarray(bs[L.i], np.float32)
        q = np.arange(128) % L.out_w_pad
        col = np.where(q < L.out_w, b[np.minimum(q, L.out_w - 1)], 0.0)
        biases[:, L.i] = col
    out = {"lhsT_head": stacks["head"].astype(BF16),
           "lhsT_rest": stacks["rest"].astype(BF16),
           "biases": biases}
    if _W_COLS["f8"]:
        out["lhsT_f8"] = stacks["f8"].astype(F8)
    return out


def replay_ids():
    """Propagate chunk-local point ids through the schedule.

    Returns [128, 2048] int array: element (p, c) of the final output
    tile holds component (p % 8) of chunk-local point ids[p, c].
    """
    ids = np.zeros((48, 2048), np.int64)
    j = np.arange(FREE)
    for g in range(16):
        for t in range(4):
            for f in range(3):
                ids[g * 3 + f, t * FREE:(t + 1) * FREE] = t * 8192 + g * FREE + j
    for L in _LAYERS:
        out = np.zeros((128, L.C_out), np.int64)
        for m in range(L.n_mms):
            h, cbi, tau, s, b = _mm_geom(L, m)
            src = ids[h * L.Kmm:h * L.Kmm + L.Kmm:L.in_w,
                      cbi * FREE:(cbi + 1) * FREE]          # [Gmm, 512]
            out[s * L.Mmm:(s + 1) * L.Mmm,
                tau * L.blk + b * FREE:tau * L.blk + (b + 1) * FREE] = \
                np.repeat(src, L.out_w_pad, axis=0)
        ids = out
    return ids


def simulate_chunk(coords_tile, weights):
    """Numpy mirror of the device program for one [48, 2048] chunk tile."""
    group_of = {"head": "lhsT_head", "f8": "lhsT_f8", "rest": "lhsT_rest"}
    biases = np.asarray(weights["biases"], np.float32)
    act = np.asarray(coords_tile, np.float32)
    for L in _LAYERS:
        stack = np.asarray(weights[group_of[L.w_group]], np.float32)
        out = np.zeros((128, L.C_out), np.float32)
        for m in range(L.n_mms):
            h, cbi, tau, s, b = _mm_geom(L, m)
            lhsT = stack[h * L.Kmm:h * L.Kmm + L.Kmm,
                         L.w_col:L.w_col + L.Mmm]
            rhs = act[h * L.Kmm:h * L.Kmm + L.Kmm,
                      cbi * FREE:(cbi + 1) * FREE]
            out[s * L.Mmm:(s + 1) * L.Mmm,
                tau * L.blk + b * FREE:tau * L.blk + (b + 1) * FREE] = \
                lhsT.T @ rhs
        out += biases[:, L.i:L.i + 1]
        act = np.tanh(out) if L.i < N_LAYERS - 1 else out
        dt = F8 if L.i in FP8_ACT_LAYERS else BF16
        act = act.astype(dt).astype(np.float32)
    return act                                   # [128, C_out(last)]


def unpack_output(per_core_out):
    """[NCORES][NCHUNKS, 128, 2048] device tiles -> [N_POINTS, 3]."""
    ids = replay_ids()
    rows = np.arange(128)
    comp = rows % _LAYERS[-1].out_w_pad
    valid = comp < 3
    n_idx = ids[valid]
    o_idx = np.broadcast_to(comp[valid][:, None], n_idx.shape)
    out = np.empty((N_POINTS, 3), np.float32)
    for core in range(NCORES):
        tiles = per_core_out[core]
        for chunk in range(NCHUNKS):
            base = core * PPC + chunk * CHUNK
            t = np.asarray(tiles[chunk], np.float32)
            out[base + n_idx, o_idx] = t[valid]
    return out


# ---------------------------------------------------------------- bass program

_PROGRAM_CACHE = {}


def _build_program(repeat=1):
    import concourse.bacc as bacc
    import concourse.tile as tile
    from concourse import mybir

    nc = bacc.Bacc("TRN2", target_bir_lowering=False, debug=False,
                   enable_asserts=False, num_devices=NCORES)
    f32 = mybir.dt.float32
    b16 = mybir.dt.bfloat16
    f8 = mybir.dt.float8e4
    coords_d = nc.dram_tensor("coords", (NCHUNKS, 48, 2048), b16,
                              kind="ExternalInput").ap()
    wh_d = nc.dram_tensor("lhsT_head", (128, _W_COLS["head"]), b16,
                          kind="ExternalInput").ap()
    wr_d = nc.dram_tensor("lhsT_rest", (128, _W_COLS["rest"]), b16,
                          kind="ExternalInput").ap()
    wf8_d = (nc.dram_tensor("lhsT_f8", (128, _W_COLS["f8"]), f8,
                            kind="ExternalInput").ap()
             if _W_COLS["f8"] else None)
    b_d = nc.dram_tensor("biases", (128, N_LAYERS), f32,
                         kind="ExternalInput").ap()
    out_d = nc.dram_tensor("out", (NCHUNKS, 128, _LAYERS[-1].C_out), b16,
                           kind="ExternalOutput").ap()

    TANH = mybir.ActivationFunctionType.Tanh
    IDENT = mybir.ActivationFunctionType.Identity

    with tile.TileContext(nc) as tc, ExitStack() as ctx:
        wpool = ctx.enter_context(tc.tile_pool(name="weights", bufs=1))
        cpool = ctx.enter_context(tc.tile_pool(name="cin", bufs=6))
        pA = ctx.enter_context(tc.tile_pool(name="a2k", bufs=4))
        pB = ctx.enter_context(tc.tile_pool(name="a4k", bufs=6))
        pC = ctx.enter_context(tc.tile_pool(name="a8k", bufs=4))
        pout = ctx.enter_context(tc.tile_pool(name="aout", bufs=4))
        pspool = ctx.enter_context(
            tc.tile_pool(name="psum", bufs=2, space="PSUM"))

        # PE p-state warmup: the tensor engine needs ~3us of continuous
        # execution to reach full clock. Run throwaway matmuls on a
        # zeroed scratch tile while the input DMAs are in flight so L0
        # hits the PE at full speed. GPSIMD does the memset (every other
        # engine has real work at t0).
        dummy = wpool.tile([128, FREE], b16, tag="warm")
        nc.gpsimd.memset(dummy[:], 0.0)
        wps = pspool.tile([128, BLK], f32, tag="ps")
        for _ in range(6):
            nc.tensor.matmul(wps[:, 0:FREE], dummy[:, 0:128], dummy[:],
                             start=True, stop=True)
        # Dummy tanh at t0: the act-table load (~1.3us) is otherwise
        # charged lazily against the first real activation. ACT is idle
        # here, so the table is hot before L0's tanh arrives.
        warm_act = wpool.tile([128, FREE], b16, tag="warmact")
        nc.scalar.activation(warm_act[:], dummy[:], TANH)

        wt_head = wpool.tile([128, _W_COLS["head"]], b16, tag="wth")
        nc.sync.dma_start(out=wt_head[:], in_=wh_d[:])
        bt = wpool.tile([128, N_LAYERS], f32, tag="bt")
        nc.sync.dma_start(out=bt[:], in_=b_d[:])

        ct_of = {}

        def fetch(c, split=1):
            t = cpool.tile([48, 2048], b16, tag="cin")
            # split>1 engages parallel DMA queues so the very first tile
            # (which gates the whole pipeline) lands sooner.
            step = 48 // split
            for j in range(split):
                nc.sync.dma_start(out=t[j * step:(j + 1) * step, :],
                                  in_=coords_d[c % NCHUNKS][j * step:(j + 1)
                                                            * step, :])
            ct_of[c] = t

        pool_by_cols = {2048: pA, 4096: pB, 8192: pC}

        def wslice(L, h):
            wt = {"head": wt_head, "f8": wt_f8, "rest": wt_rest}[L.w_group]
            return wt[h * L.Kmm:(h + 1) * L.Kmm, L.w_col:L.w_col + L.Mmm]

        def emit_layer(L, act):
            is_last = L.i == N_LAYERS - 1
            pool = pout if is_last else pool_by_cols[L.C_out]
            out_dt = f8 if L.i in FP8_ACT_LAYERS else b16
            out_t = pool.tile([128, L.C_out], out_dt, tag=pool.name)
            for tau in range(L.n_mega):
                ps = pspool.tile([128, L.blk], f32, tag="ps")
                for r in range(L.banks * L.stack):
                    m = tau * L.banks * L.stack + r
                    h, cbi, _, s, b = _mm_geom(L, m)
                    nc.tensor.matmul(
                        ps[s * L.Mmm:(s + 1) * L.Mmm,
                           b * FREE:(b + 1) * FREE],
                        wslice(L, h),
                        act[h * L.Kmm:(h + 1) * L.Kmm,
                            cbi * FREE:(cbi + 1) * FREE],
                        start=True, stop=True,
                        # auto-derive rejects the 4th col slot (96)
                        tile_position=(h * L.Kmm % 128, s * L.Mmm % 128))
                dst = out_t[:, tau * L.blk:(tau + 1) * L.blk]
                if is_last and L14_ON_DVE:
                    # Two halves so each releases its PSUM banks sooner,
                    # softening the PE->DVE->PE slot-return stall.
                    half = L.blk // 2
                    for j in range(2):
                        nc.vector.tensor_scalar_add(
                            dst[:, j * half:(j + 1) * half],
                            ps[:, j * half:(j + 1) * half],
                            bt[:, L.i:L.i + 1])
                elif is_last:
                    nc.scalar.activation(
                        dst, ps[:], IDENT, bias=bt[:, L.i:L.i + 1])
                else:
                    nc.scalar.activation(
                        dst, ps[:], TANH, bias=bt[:, L.i:L.i + 1])
            return out_t

        def emit_tail(c, acts, split=False):
            if not split:
                out_t = emit_layer(_LAYERS[-1], acts[c])
                nc.sync.dma_start(out=out_d[c % NCHUNKS], in_=out_t[:])
                return
            # Program-final tails: drain the single L14 mega in halves
            # into separate tiles so the first half's store DMA overlaps
            # the second half's drain.
            L = _LAYERS[-1]
            ps = pspool.tile([128, L.blk], f32, tag="ps")
            for m in range(L.banks * L.stack):
                h, cbi, _, s, b = _mm_geom(L, m)
                nc.tensor.matmul(
                    ps[s * L.Mmm:(s + 1) * L.Mmm, b * FREE:(b + 1) * FREE],
                    wslice(L, h),
                    acts[c][h * L.Kmm:(h + 1) * L.Kmm,
                            cbi * FREE:(cbi + 1) * FREE],
                    start=True, stop=True,
                    tile_position=(h * L.Kmm % 128, s * L.Mmm % 128))
            half = L.blk // 2
            for j in range(2):
                ot = pout.tile([128, half], b16, tag="aout")
                if L14_ON_DVE:
                    nc.vector.tensor_scalar_add(
                        ot[:], ps[:, j * half:(j + 1) * half],
                        bt[:, L.i:L.i + 1])
                else:
                    nc.scalar.activation(
                        ot[:], ps[:, j * half:(j + 1) * half], IDENT,
                        bias=bt[:, L.i:L.i + 1])
                nc.sync.dma_start(
                    out=out_d[c % NCHUNKS][:, j * half:(j + 1) * half],
                    in_=ot[:])

        seq = [r * NCHUNKS + c for r in range(repeat) for c in range(NCHUNKS)]
        pairs = list(zip(seq[0::2], seq[1::2]))

        # Coords for the first two pairs land before the weight bulk so
        # L0 can start as early as possible. fp8 weights (early layers)
        # ship right after the first coords tiles.
        fetch(pairs[0][0], split=2)
        if _W_COLS["f8"]:
            wt_f8 = wpool.tile([128, _W_COLS["f8"]], f8, tag="wtf8")
            nc.sync.dma_start(out=wt_f8[:], in_=wf8_d[:])
        else:
            wt_f8 = None
        fetch(pairs[0][1])
        wt_rest = wpool.tile([128, _W_COLS["rest"]], b16, tag="wtr")
        nc.sync.dma_start(out=wt_rest[:], in_=wr_d[:])
        if len(pairs) > 1:
            fetch(pairs[1][0])
            fetch(pairs[1][1])

        acts = {}
        prev = None
        for pi, (ca, cb) in enumerate(pairs):
            is_last_pair = pi == len(pairs) - 1
            first_li = 0
            if pi == 0:
                acts[ca] = ct_of.pop(ca)
                acts[cb] = ct_of.pop(cb)
            else:
                first_li = 1      # L0 was pre-emitted by the previous pair
            for li in range(first_li, N_LAYERS - 1):
                L = _LAYERS[li]
                for ci, c in enumerate((ca, cb)):
                    acts[c] = emit_layer(L, acts[c])
                    if li == N_LAYERS - 2 and not is_last_pair:
                        # Keep ACT fed across the pair boundary: the
                        # next pair's L0 goes onto the PE queue now.
                        n = pairs[pi + 1][ci]
                        acts[n] = emit_layer(_LAYERS[0], ct_of.pop(n))
                if li == N_LAYERS - 2 and is_last_pair:
                    for c in (ca, cb):
                        emit_tail(c, acts, split=True)
                if li == 1:
                    # ACT is busy with L0/L1 tanh here; slot the previous
                    # pair's (ACT-free) final layer into the PE stream now
                    # so pair transitions cost ACT nothing.
                    if prev is not None:
                        for c in prev:
                            emit_tail(c, acts)
                    if pi + 2 < len(pairs):
                        fetch(pairs[pi + 2][0])
                        fetch(pairs[pi + 2][1])
            prev = (ca, cb)

    nc.compile()
    return nc


def get_program(repeat=1):
    key = ("nc", repeat)
    if key not in _PROGRAM_CACHE:
        _PROGRAM_CACHE[key] = _build_program(repeat)
    return _PROGRAM_CACHE[key]


def make_in_maps(coords, Ws, bs):
    cp = pack_coords(coords)
    weights = build_weights(Ws, bs)
    return [{"coords": cp[core], **weights} for core in range(NCORES)]


def kernel(**inputs):
    from concourse.bass_utils import run_bass_kernel_spmd

    coords = np.asarray(inputs["coords"], np.float32)
    Ws = [np.asarray(inputs[f"W{i}"], np.float32) for i in range(N_LAYERS)]
    bs = [np.asarray(inputs[f"b{i}"], np.float32) for i in range(N_LAYERS)]

    nc = get_program()
    in_maps = make_in_maps(coords, Ws, bs)
    res = run_bass_kernel_spmd(nc, in_maps, list(range(NCORES)))
    per_core = [res.results[c]["out"] for c in range(NCORES)]
    full = unpack_output(per_core)
    return (full[:, 0:1], full[:, 1:2], full[:, 2:3])



# revision 33
# speedup vs baseline: 1.2931x; 1.1514x over previous
"""Trainium2 Bass kernel for ClassicPINN forward pass (15-layer tiny MLP, tanh).

Strategy (v3)
-------------
v3 on top of v2 (HW-microbenchmarked): the pipeline is bound by the
ScalarE ACTIVATE stream, whose cost is ~(FD/2)/1.2GHz + ~0.45us fixed
per instruction. With PSUM capping tiles at [128, 2048] x 2, the
instruction count is fixed, so v3 cuts per-instruction cost instead:
  * Activations of L0-L10 (and the matching L1-L11 weights) are
    fp8e4m3: an fp8-out ACTIVATE measures ~0.19us faster per
    [128,2048] drain, and the early layers' quantization noise
    contracts away through the remaining stack (full-1M numpy:
    6.26e-3 vs 3.0e-3 pure-bf16, tol 2e-2; +L11 hits 1.8e-2 and is
    rejected).
  * The final 16->3 layer packs its output 4-wide (pad 1) instead of
    8-wide: stack=4 col-tiled matmuls (explicit tile_position - the
    auto-derive rejects base partition 96), a [128,1024] 2-bank mega,
    half the drain cols and half the output-store bytes of v2.
  * L14_ON_DVE optionally moves that tanh-free drain to the idle DVE;
    it defaults off - the PE->DVE->PE PSUM-slot chain it creates can
    starve ACT at pair boundaries (the v1 regression) and the A/B was
    within timing noise.

Strategy (v2)
-------------
Pure data parallel over 8 NeuronCores (131072 points each); 4 chunks of
32768 points per core. Activations live feature-on-partition in bf16:
the 128 SBUF partitions hold G groups of the layer width, each group a
different 512-point column block. Weights are block-diagonalized on the
host so every matmul is dense [K<=128, M<=128] x [K, 512] -> PSUM fp32.

ACT (ScalarE tanh) is the roofline (~216 tanh elems/point). v2 cuts ACT
waste and idle vs v1:
  * bf16 matmul dataflow (same 1 cyc/row PE speed as fp32r) allows PE
    tile_position, so layers with Mmm=64 (L13, L14) stack 2 matmuls
    partition-wise in one PSUM bank set -> every ACT drains a full
    [128, 2048] tile (v1 ran half-width ACTs for L13).
  * L14 reads the *folded* L13 tile with Gmm=8 (was 4): half the
    matmuls and half the DVE drain of v1.
  * All 15 layers run in interleaved chunk pairs (bf16 halves SBUF so
    the wide layers fit); each pair's L14 is deferred into the next
    pair's early layers and the next pair's L0 is pre-emitted during
    L13, so ACT never waits on a pair transition.
  * The L14 bias-add runs on ACT as Identity (every tanh act-table set
    contains identity, so no table reload): a DVE drain would expose a
    cross-engine PE->DVE->PE PSUM-reuse stall on ACT's critical path.
  * Coords prefetched a pair ahead; weight DMA split so L0/L1 weights
    land before the bulk; bf16 output halves the out-DMA; dummy matmuls
    at t0 ramp the PE p-state while the first DMAs are in flight; the
    program-final L14s drain in halves so the store DMA overlaps.

The same schedule object drives the Bass builder, a numpy simulator
(used by test.py), and an integer "point id" replay that yields the
output unpack permutation.
"""

import numpy as np
from contextlib import ExitStack
import ml_dtypes

BF16 = ml_dtypes.bfloat16
F8 = ml_dtypes.float8_e4m3fn

WIDTHS = [3, 8, 8, 8, 8, 8, 8, 8, 16, 16, 16, 32, 32, 32, 16, 3]
N_LAYERS = 15
N_POINTS = 1048576
NCORES = 8
PPC = N_POINTS // NCORES          # 131072 points per core
NCHUNKS = 4
CHUNK = PPC // NCHUNKS            # 32768 points per chunk
FREE = 512                        # matmul moving free dim
BANKS = 4                         # PSUM banks per mega-tile / ACT
BLK = BANKS * FREE                # 2048 cols per mega-tile
WT_HEAD_LAYERS = 2                # layers whose weights ship in the early DMA
# Layers whose tanh output is stored as fp8e4m3 (and whose consumers'
# weights are fp8): the early narrow layers tolerate it freely (numpy
# check: rel err 2.7e-3 vs 3.0e-3 pure-bf16) and fp8 ACT output may run
# the ScalarE drain at 4x. Empty set disables the path.
FP8_ACT_LAYERS = frozenset(range(11))
# Drain the final (tanh-free) layer on DVE instead of ACT. Saves ~0.8us
# of ACT per chunk but inserts a PE->DVE->PE PSUM-slot chain that can
# starve ACT at pair boundaries (the v1 regression); measure both.
L14_ON_DVE = False


class _Layer:
    pass


def _make_schedule():
    """Uniform per-layer schedule. Every layer's input is a physical
    [P_in, C_in] tile holding G_in groups of in_w features per column
    (stacked layers interleave two virtual column blocks row-wise, which
    simply doubles the group count seen by the next layer)."""
    layers = []
    G_in, C_in, P_in = 16, 2048, 48   # coords: 16 groups x 3 feats
    w_off = 0
    for i in range(N_LAYERS):
        in_w, out_w = WIDTHS[i], WIDTHS[i + 1]
        out_w_pad = 4 if out_w == 3 else out_w
        L = _Layer()
        L.i, L.in_w, L.out_w, L.out_w_pad = i, in_w, out_w, out_w_pad
        L.G_in, L.C_in, L.P_in = G_in, C_in, P_in
        L.Gmm = min(G_in, 128 // out_w_pad)
        L.n_half = G_in // L.Gmm
        L.Kmm = L.Gmm * in_w
        L.Mmm = L.Gmm * out_w_pad
        L.stack = 128 // L.Mmm        # matmuls stacked per PSUM bank set
        L.ncb = C_in // FREE
        L.n_mms = L.ncb * L.n_half
        # Last layer packs 4-wide output groups 4-deep per bank: its one
        # mega-tile spans only 2 banks ([128, 1024]).
        L.banks = min(BANKS, L.n_mms // L.stack)
        L.blk = L.banks * FREE
        assert L.n_mms % (L.banks * L.stack) == 0, (i, L.n_mms, L.stack)
        L.n_mega = L.n_mms // (L.banks * L.stack)
        L.C_out = L.n_mega * L.blk
        L.w_off = w_off
        w_off += L.Mmm
        layers.append(L)
        G_in, C_in, P_in = 128 // out_w_pad, L.C_out, 128
    return layers, w_off


_LAYERS, W_TOTAL = _make_schedule()


def _assign_weight_groups():
    """Partition layer weights into dram tensors by dtype/urgency.

    Layer i's weights must match its input dtype: fp8 iff layer i-1's
    activation output is fp8. Groups: "head" (early bf16, L0/L1),
    "f8" (all fp8 layers, also shipped early), "rest" (late bf16).
    """
    cols = {"head": 0, "f8": 0, "rest": 0}
    for L in _LAYERS:
        fp8_w = (L.i - 1) in FP8_ACT_LAYERS
        if fp8_w:
            L.w_group = "f8"
        elif L.i < WT_HEAD_LAYERS:
            L.w_group = "head"
        else:
            L.w_group = "rest"
        L.w_col = cols[L.w_group]
        cols[L.w_group] += L.Mmm
    return cols


_W_COLS = _assign_weight_groups()


def _mm_geom(L, m):
    """Matmul m -> (input half, input col block, mega-tile, stack row, bank)."""
    h, cbi = divmod(m, L.ncb)
    tau, r = divmod(m, L.banks * L.stack)
    s, b = divmod(r, L.banks)
    return h, cbi, tau, s, b


# ---------------------------------------------------------------- host packing

def pack_coords(coords):
    """[N_POINTS, 3] -> bf16 [NCORES, NCHUNKS, 48, 2048] matching L0 layout.

    Per core: point n = chunk*32768 + t*8192 + g*512 + j lives at
    partition g*3+f, column t*512+j of tile [core, chunk].
    """
    c = np.ascontiguousarray(coords, dtype=np.float32)
    c = c.reshape(NCORES, NCHUNKS, 4, 16, FREE, 3)
    c = c.transpose(0, 1, 3, 5, 2, 4)            # core, chunk, g, f, t, j
    return np.ascontiguousarray(c.reshape(NCORES, NCHUNKS, 48, 2048)).astype(BF16)


def build_weights(Ws, bs):
    """Block-diagonal lhsT stacks per dtype group and bias matrix [128, 15]."""
    stacks = {k: np.zeros((128, n), np.float32) for k, n in _W_COLS.items()}
    biases = np.zeros((128, N_LAYERS), np.float32)
    for L in _LAYERS:
        W = np.asarray(Ws[L.i], np.float32)      # [out_w, in_w]
        bd = np.zeros((L.Kmm, L.Mmm), np.float32)
        for g in range(L.Gmm):
            bd[g * L.in_w:(g + 1) * L.in_w,
               g * L.out_w_pad:g * L.out_w_pad + L.out_w] = W.T
        for h in range(L.n_half):
            base = h * L.Kmm
            stacks[L.w_group][base:base + L.Kmm,
                              L.w_col:L.w_col + L.Mmm] = bd
        b = np.asarray(bs[L.i], np.float32)
        q = np.arange(128) % L.out_w_pad
        col = np.where(q < L.out_w, b[np.minimum(q, L.out_w - 1)], 0.0)
        biases[:, L.i] = col
    out = {"lhsT_head": stacks["head"].astype(BF16),
           "lhsT_rest": stacks["rest"].astype(BF16),
           "biases": biases}
    if _W_COLS["f8"]:
        out["lhsT_f8"] = stacks["f8"].astype(F8)
    return out


def replay_ids():
    """Propagate chunk-local point ids through the schedule.

    Returns [128, 2048] int array: element (p, c) of the final output
    tile holds component (p % 8) of chunk-local point ids[p, c].
    """
    ids = np.zeros((48, 2048), np.int64)
    j = np.arange(FREE)
    for g in range(16):
        for t in range(4):
            for f in range(3):
                ids[g * 3 + f, t * FREE:(t + 1) * FREE] = t * 8192 + g * FREE + j
    for L in _LAYERS:
        out = np.zeros((128, L.C_out), np.int64)
        for m in range(L.n_mms):
            h, cbi, tau, s, b = _mm_geom(L, m)
            src = ids[h * L.Kmm:h * L.Kmm + L.Kmm:L.in_w,
                      cbi * FREE:(cbi + 1) * FREE]          # [Gmm, 512]
            out[s * L.Mmm:(s + 1) * L.Mmm,
                tau * L.blk + b * FREE:tau * L.blk + (b + 1) * FREE] = \
                np.repeat(src, L.out_w_pad, axis=0)
        ids = out
    return ids


def simulate_chunk(coords_tile, weights):
    """Numpy mirror of the device program for one [48, 2048] chunk tile."""
    group_of = {"head": "lhsT_head", "f8": "lhsT_f8", "rest": "lhsT_rest"}
    biases = np.asarray(weights["biases"], np.float32)
    act = np.asarray(coords_tile, np.float32)
    for L in _LAYERS:
        stack = np.asarray(weights[group_of[L.w_group]], np.float32)
        out = np.zeros((128, L.C_out), np.float32)
        for m in range(L.n_mms):
            h, cbi, tau, s, b = _mm_geom(L, m)
            lhsT = stack[h * L.Kmm:h * L.Kmm + L.Kmm,
                         L.w_col:L.w_col + L.Mmm]
            rhs = act[h * L.Kmm:h * L.Kmm + L.Kmm,
                      cbi * FREE:(cbi + 1) * FREE]
            out[s * L.Mmm:(s + 1) * L.Mmm,
                tau * L.blk + b * FREE:tau * L.blk + (b + 1) * FREE] = \
                lhsT.T @ rhs
        out += biases[:, L.i:L.i + 1]
        act = np.tanh(out) if L.i < N_LAYERS - 1 else out
        dt = F8 if L.i in FP8_ACT_LAYERS else BF16
        act = act.astype(dt).astype(np.float32)
    return act                                   # [128, C_out(last)]


def unpack_output(per_core_out):
    """[NCORES][NCHUNKS, 128, 2048] device tiles -> [N_POINTS, 3]."""
    ids = replay_ids()
    rows = np.arange(128)
    comp = rows % _LAYERS[-1].out_w_pad
    valid = comp < 3
    n_idx = ids[valid]
    o_idx = np.broadcast_to(comp[valid][:, None], n_idx.shape)
    out = np.empty((N_POINTS, 3), np.float32)
    for core in range(NCORES):
        tiles = per_core_out[core]
        for chunk in range(NCHUNKS):
            base = core * PPC + chunk * CHUNK
            t = np.asarray(tiles[chunk], np.float32)
            out[base + n_idx, o_idx] = t[valid]
    return out


# ---------------------------------------------------------------- bass program

_PROGRAM_CACHE = {}


def _build_program(repeat=1):
    import concourse.bacc as bacc
    import concourse.tile as tile
    from concourse import mybir

    nc = bacc.Bacc("TRN2", target_bir_lowering=False, debug=False,
                   enable_asserts=False, num_devices=NCORES)
    f32 = mybir.dt.float32
    b16 = mybir.dt.bfloat16
    f8 = mybir.dt.float8e4
    coords_d = nc.dram_tensor("coords", (NCHUNKS, 48, 2048), b16,
                              kind="ExternalInput").ap()
    wh_d = nc.dram_tensor("lhsT_head", (128, _W_COLS["head"]), b16,
                          kind="ExternalInput").ap()
    wr_d = nc.dram_tensor("lhsT_rest", (128, _W_COLS["rest"]), b16,
                          kind="ExternalInput").ap()
    wf8_d = (nc.dram_tensor("lhsT_f8", (128, _W_COLS["f8"]), f8,
                            kind="ExternalInput").ap()
             if _W_COLS["f8"] else None)
    b_d = nc.dram_tensor("biases", (128, N_LAYERS), f32,
                         kind="ExternalInput").ap()
    out_d = nc.dram_tensor("out", (NCHUNKS, 128, _LAYERS[-1].C_out), b16,
                           kind="ExternalOutput").ap()

    TANH = mybir.ActivationFunctionType.Tanh
    IDENT = mybir.ActivationFunctionType.Identity

    with tile.TileContext(nc) as tc, ExitStack() as ctx:
        wpool = ctx.enter_context(tc.tile_pool(name="weights", bufs=1))
        cpool = ctx.enter_context(tc.tile_pool(name="cin", bufs=6))
        pA = ctx.enter_context(tc.tile_pool(name="a2k", bufs=4))
        pB = ctx.enter_context(tc.tile_pool(name="a4k", bufs=6))
        pC = ctx.enter_context(tc.tile_pool(name="a8k", bufs=4))
        pout = ctx.enter_context(tc.tile_pool(name="aout", bufs=4))
        pspool = ctx.enter_context(
            tc.tile_pool(name="psum", bufs=2, space="PSUM"))

        # PE p-state warmup: the tensor engine needs ~3us of continuous
        # execution to reach full clock. Run throwaway matmuls on a
        # zeroed scratch tile while the input DMAs are in flight so L0
        # hits the PE at full speed. GPSIMD does the memset (every other
        # engine has real work at t0).
        dummy = wpool.tile([128, FREE], b16, tag="warm")
        nc.gpsimd.memset(dummy[:], 0.0)
        wps = pspool.tile([128, BLK], f32, tag="ps")
        for _ in range(6):
            nc.tensor.matmul(wps[:, 0:FREE], dummy[:, 0:128], dummy[:],
                             start=True, stop=True)
        # Dummy tanh at t0: the act-table load (~1.3us) is otherwise
        # charged lazily against the first real activation. ACT is idle
        # here, so the table is hot before L0's tanh arrives.
        warm_act = wpool.tile([128, FREE], b16, tag="warmact")
        nc.scalar.activation(warm_act[:], dummy[:], TANH)

        wt_head = wpool.tile([128, _W_COLS["head"]], b16, tag="wth")
        nc.sync.dma_start(out=wt_head[:], in_=wh_d[:])
        bt = wpool.tile([128, N_LAYERS], f32, tag="bt")
        nc.sync.dma_start(out=bt[:], in_=b_d[:])

        ct_of = {}

        def fetch(c, split=1):
            t = cpool.tile([48, 2048], b16, tag="cin")
            # split>1 engages parallel DMA queues so the very first tile
            # (which gates the whole pipeline) lands sooner.
            step = 48 // split
            for j in range(split):
                nc.sync.dma_start(out=t[j * step:(j + 1) * step, :],
                                  in_=coords_d[c % NCHUNKS][j * step:(j + 1)
                                                            * step, :])
            ct_of[c] = t

        pool_by_cols = {2048: pA, 4096: pB, 8192: pC}

        def wslice(L, h):
            wt = {"head": wt_head, "f8": wt_f8, "rest": wt_rest}[L.w_group]
            return wt[h * L.Kmm:(h + 1) * L.Kmm, L.w_col:L.w_col + L.Mmm]

        def emit_layer(L, act):
            is_last = L.i == N_LAYERS - 1
            pool = pout if is_last else pool_by_cols[L.C_out]
            out_dt = f8 if L.i in FP8_ACT_LAYERS else b16
            out_t = pool.tile([128, L.C_out], out_dt, tag=pool.name)
            for tau in range(L.n_mega):
                ps = pspool.tile([128, L.blk], f32, tag="ps")
                for r in range(L.banks * L.stack):
                    m = tau * L.banks * L.stack + r
                    h, cbi, _, s, b = _mm_geom(L, m)
                    nc.tensor.matmul(
                        ps[s * L.Mmm:(s + 1) * L.Mmm,
                           b * FREE:(b + 1) * FREE],
                        wslice(L, h),
                        act[h * L.Kmm:(h + 1) * L.Kmm,
                            cbi * FREE:(cbi + 1) * FREE],
                        start=True, stop=True,
                        # auto-derive rejects the 4th col slot (96)
                        tile_position=(h * L.Kmm % 128, s * L.Mmm % 128))
                dst = out_t[:, tau * L.blk:(tau + 1) * L.blk]
                if is_last and L14_ON_DVE:
                    # Two halves so each releases its PSUM banks sooner,
                    # softening the PE->DVE->PE slot-return stall.
                    half = L.blk // 2
                    for j in range(2):
                        nc.vector.tensor_scalar_add(
                            dst[:, j * half:(j + 1) * half],
                            ps[:, j * half:(j + 1) * half],
                            bt[:, L.i:L.i + 1])
                elif is_last:
                    nc.scalar.activation(
                        dst, ps[:], IDENT, bias=bt[:, L.i:L.i + 1])
                else:
                    nc.scalar.activation(
                        dst, ps[:], TANH, bias=bt[:, L.i:L.i + 1])
            return out_t

        def emit_tail(c, acts, split=False):
            if not split:
                out_t = emit_layer(_LAYERS[-1], acts[c])
                nc.sync.dma_start(out=out_d[c % NCHUNKS], in_=out_t[:])
                return
            # Program-final tails: drain the single L14 mega in halves
            # into separate tiles so the first half's store DMA overlaps
            # the second half's drain.
            L = _LAYERS[-1]
            ps = pspool.tile([128, L.blk], f32, tag="ps")
            for m in range(L.banks * L.stack):
                h, cbi, _, s, b = _mm_geom(L, m)
                nc.tensor.matmul(
                    ps[s * L.Mmm:(s + 1) * L.Mmm, b * FREE:(b + 1) * FREE],
                    wslice(L, h),
                    acts[c][h * L.Kmm:(h + 1) * L.Kmm,
                            cbi * FREE:(cbi + 1) * FREE],
                    start=True, stop=True,
                    tile_position=(h * L.Kmm % 128, s * L.Mmm % 128))
            half = L.blk // 2
            for j in range(2):
                ot = pout.tile([128, half], b16, tag="aout")
                if L14_ON_DVE:
                    nc.vector.tensor_scalar_add(
                        ot[:], ps[:, j * half:(j + 1) * half],
                        bt[:, L.i:L.i + 1])
                else:
                    nc.scalar.activation(
                        ot[:], ps[:, j * half:(j + 1) * half], IDENT,
                        bias=bt[:, L.i:L.i + 1])
                nc.sync.dma_start(
                    out=out_d[c % NCHUNKS][:, j * half:(j + 1) * half],
                    in_=ot[:])

        seq = [r * NCHUNKS + c for r in range(repeat) for c in range(NCHUNKS)]
        pairs = list(zip(seq[0::2], seq[1::2]))

        # Coords for the first two pairs land before the weight bulk so
        # L0 can start as early as possible. fp8 weights (early layers)
        # ship right after the first coords tiles.
        fetch(pairs[0][0], split=2)
        if _W_COLS["f8"]:
            wt_f8 = wpool.tile([128, _W_COLS["f8"]], f8, tag="wtf8")
            nc.sync.dma_start(out=wt_f8[:], in_=wf8_d[:])
        else:
            wt_f8 = None
        fetch(pairs[0][1])
        wt_rest = wpool.tile([128, _W_COLS["rest"]], b16, tag="wtr")
        nc.sync.dma_start(out=wt_rest[:], in_=wr_d[:])
        if len(pairs) > 1:
            fetch(pairs[1][0])
            fetch(pairs[1][1])

        acts = {}
        prev = None
        for pi, (ca, cb) in enumerate(pairs):
            is_last_pair = pi == len(pairs) - 1
            first_li = 0
            if pi == 0:
                acts[ca] = ct_of.pop(ca)
                acts[cb] = ct_of.pop(cb)
            else:
                first_li = 1      # L0 was pre-emitted by the previous pair
            for li in range(first_li, N_LAYERS - 1):
                L = _LAYERS[li]
                for ci, c in enumerate((ca, cb)):
                    acts[c] = emit_layer(L, acts[c])
                    if li == N_LAYERS - 2 and not is_last_pair:
                        # Keep ACT fed across the pair boundary: the
                        # next pair's L0 goes onto the PE queue now.
                        n = pairs[pi + 1][ci]
                        acts[n] = emit_layer(_LAYERS[0], ct_of.pop(n))
                if li == N_LAYERS - 2 and is_last_pair:
                    for c in (ca, cb):
                        emit_tail(c, acts, split=True)
                if li == 1:
                    # ACT is busy with L0/L1 tanh here; slot the previous
                    # pair's (ACT-free) final layer into the PE stream now
                    # so pair transitions cost ACT nothing.
                    if prev is not None:
                        for c in prev:
                            emit_tail(c, acts)
                    if pi + 2 < len(pairs):
                        fetch(pairs[pi + 2][0])
                        fetch(pairs[pi + 2][1])
            prev = (ca, cb)

    nc.compile()
    return nc


def get_program(repeat=1):
    key = ("nc", repeat)
    if key not in _PROGRAM_CACHE:
        _PROGRAM_CACHE[key] = _build_program(repeat)
    return _PROGRAM_CACHE[key]


def make_in_maps(coords, Ws, bs):
    cp = pack_coords(coords)
    weights = build_weights(Ws, bs)
    return [{"coords": cp[core], **weights} for core in range(NCORES)]


def kernel(**inputs):
    from concourse.bass_utils import run_bass_kernel_spmd

    coords = np.asarray(inputs["coords"], np.float32)
    Ws = [np.asarray(inputs[f"W{i}"], np.float32) for i in range(N_LAYERS)]
    bs = [np.asarray(inputs[f"b{i}"], np.float32) for i in range(N_LAYERS)]

    nc = get_program()
    in_maps = make_in_maps(coords, Ws, bs)
    res = run_bass_kernel_spmd(nc, in_maps, list(range(NCORES)))
    per_core = [res.results[c]["out"] for c in range(NCORES)]
    full = unpack_output(per_core)
    return (full[:, 0:1], full[:, 1:2], full[:, 2:3])

